# revision 1
# baseline (speedup 1.0000x reference)
"""AttentionWithRope Trainium2 Bass kernel.

Sharding: 8 cores = 2 batches x 4 head-groups (4 heads / 256 features each).
Each core computes q/k/v projections for its feature slice on its batch,
RoPE, causal attention for its 4 heads, and a partial output projection
(columns of attn_out vs rows of wo). The host sums the 4 partials per batch
(the tensor-parallel all-reduce) to produce the full output.

Device program (per core), all fp32 with float32r matmuls:
  phase A: load x^T; q^T/k^T = W^T.T @ x^T (+bias via ACT evac); RoPE on DVE;
           v token-major [2048, 260] with fused bias + ones-col + padding mask.
  phase B: per head, per q-chunk(512): scores^T tiles [k=128, q<=512] on PE,
           exp on ACT (causal-sliced), diagonal tri-mask on DVE,
           z^T [65, 512] accumulated on PE (ones row -> softmax sums).
  phase C: reciprocal of sums, normalize z^T via DRAM-broadcast multiply,
           out = zn^T.T @ wo^T (+wo_b via K=1 matmul) -> psum -> DMA to DRAM.
"""

import numpy as np
from contextlib import ExitStack

DIM, HEADS, HD = 1024, 16, 64
B, S = 2, 2048
NC = 8
HPC = 4          # heads per core
F = HPC * HD     # 256 features per core
ROPE_BASE = 10000.0


def _rope_tables():
    theta = ROPE_BASE ** (-np.arange(0, HD, 2, dtype=np.float32) / HD)  # [32]
    pos = np.arange(S, dtype=np.float32)
    ang = pos[:, None] * theta[None, :]          # [S, 32]
    cos, sin = np.cos(ang).T, np.sin(ang).T      # [32, S]
    CT = np.concatenate([cos, cos, cos, cos], 0).astype(np.float32)      # [128, S]
    SST = np.concatenate([-sin, sin, -sin, sin], 0).astype(np.float32)   # [128, S]
    return CT, SST


def _build_program(debug=False):
    import concourse.bass as bass
    import concourse.mybir as mybir
    import concourse.tile as tile
    from concourse import bacc

    fp32 = mybir.dt.float32
    f32r = mybir.dt.float32r
    AF = mybir.ActivationFunctionType
    ALU = mybir.AluOpType

    nc = bacc.Bacc("TRN2", target_bir_lowering=False, num_devices=NC)

    from bass_rust import add_dep_helper as _adh
    _prev_mm = [None]

    def MM(*args, **kw):
        bi = nc.tensor.matmul(*args, **kw)
        if _prev_mm[0] is not None:
            _adh(bi.ins, _prev_mm[0].ins, sync=False, reason="pe-order")
        _prev_mm[0] = bi
        return bi

    # ---- DRAM I/O ----
    xT_d = nc.dram_tensor("xT", [DIM + 128, S], f32r, kind="ExternalInput").ap()
    wqT_d = nc.dram_tensor("wqT", [DIM, F], f32r, kind="ExternalInput").ap()
    wkT_d = nc.dram_tensor("wkT", [DIM, F], f32r, kind="ExternalInput").ap()
    wvTe_d = nc.dram_tensor("wvTe", [DIM + 128, 260], f32r, kind="ExternalInput").ap()
    woT_d = nc.dram_tensor("woT", [F, DIM], f32r, kind="ExternalInput").ap()
    qb_d = nc.dram_tensor("qb", [F, 1], fp32, kind="ExternalInput").ap()
    kb_d = nc.dram_tensor("kb", [F, 1], fp32, kind="ExternalInput").ap()
    wob_d = nc.dram_tensor("wob", [128, DIM], f32r, kind="ExternalInput").ap()
    znb_d = nc.dram_tensor("znb", [128, 128], f32r, kind="ExternalInput").ap()
    mask_d = nc.dram_tensor("maskv", [S, 1], fp32, kind="ExternalInput").ap()
    CT_d = nc.dram_tensor("CT", [128, S], fp32, kind="ExternalInput").ap()
    SST_d = nc.dram_tensor("SST", [128, S], fp32, kind="ExternalInput").ap()
    tri_d = nc.dram_tensor("tri", [128, 128], fp32, kind="ExternalInput").ap()
    out_d = nc.dram_tensor("outp", [S, DIM], fp32, kind="ExternalOutput").ap()
    if debug:
        dbg_qT = nc.dram_tensor("dbg_qT", [2, 128, S], f32r, kind="ExternalOutput").ap()
        dbg_kT = nc.dram_tensor("dbg_kT", [2, 128, S], f32r, kind="ExternalOutput").ap()
        dbg_v = nc.dram_tensor("dbg_v", [128, 16 * 260], f32r, kind="ExternalOutput").ap()
        dbg_zn = nc.dram_tensor("dbg_zn", [2, 128, S], f32r, kind="ExternalOutput").ap()
        dbg_sums = nc.dram_tensor("dbg_sums", [16, 512], fp32, kind="ExternalOutput").ap()
        dbg_sumsb = nc.dram_tensor("dbg_sumsb", [1, 16 * 512], fp32, kind="ExternalOutput").ap()
        dbg_rbc = nc.dram_tensor("dbg_rbc", [128, 512], fp32, kind="ExternalOutput").ap()

    with tile.TileContext(nc) as tc, ExitStack() as ctx:
        # ---------- persistent SBUF ----------
        const = ctx.enter_context(tc.tile_pool(name="const", bufs=1))
        qk_pool = ctx.enter_context(tc.tile_pool(name="qk", bufs=1))
        v_pool = ctx.enter_context(tc.tile_pool(name="v", bufs=1))

        tri_s = const.tile([128, 128], fp32, tag="tri", name="tri")
        qb_s = const.tile([128, 2], fp32, tag="qb", name="qb")
        kb_s = const.tile([128, 2], fp32, tag="kb", name="kb")
        mask_s = const.tile([128, 16], fp32, tag="maskv", name="maskv")
        woT_s = [const.tile([128, DIM], f32r, tag=f"woT{t}", name=f"woT{t}") for t in range(2)]
        znb_s = const.tile([128, 128], f32r, tag="znb", name="znb")
        wobx_s = const.tile([128, DIM], f32r, tag="wobx", name="wobx")
        nc.sync.dma_start(znb_s[:], znb_d[:])
        nc.sync.dma_start(tri_s[:], tri_d[:])
        nc.sync.dma_start(qb_s[:], qb_d.rearrange("(c p) one -> p (c one)", p=128))
        nc.sync.dma_start(kb_s[:], kb_d.rearrange("(c p) one -> p (c one)", p=128))
        nc.sync.dma_start(wobx_s[:], wob_d[:])
        # mask [S] -> [128, 16] (partition-major chunks)
        nc.sync.dma_start(mask_s[:], mask_d.rearrange("(t p) one -> p (t one)", p=128))
        for t in range(2):
            nc.sync.dma_start(woT_s[t][:], woT_d[128 * t:128 * t + 128, :])

        qhatT = [qk_pool.tile([128, S], f32r, tag=f"qhatT{t}", name=f"qhatT{t}") for t in range(2)]
        khatT = [qk_pool.tile([128, S], f32r, tag=f"khatT{t}", name=f"khatT{t}") for t in range(2)]
        v_big = v_pool.tile([128, 16 * 260], f32r, tag="vbig", name="vbig")
        v_sb = [v_big[:, 260 * t:260 * t + 260] for t in range(16)]

        # ---------- phase A: projections + rope ----------
        with tc.tile_pool(name="xT", bufs=1) as xp, \
             tc.tile_pool(name="wqk", bufs=1) as wp, \
             tc.tile_pool(name="ppsum", bufs=3, space="PSUM") as pp, \
             tc.tile_pool(name="ropetmp", bufs=2) as rp:
            CT_s = xp.tile([128, S], fp32, tag="CT", name="CT")
            SST_s = xp.tile([128, S], fp32, tag="SST", name="SST")
            nc.sync.dma_start(CT_s[:], CT_d[:])
            nc.sync.dma_start(SST_s[:], SST_d[:])
            xT_big = xp.tile([128, 9 * S], f32r, tag="xTbig", name="xTbig")
            xT_s = [xT_big[:, S * d:S * d + S] for d in range(9)]
            for d in range(9):
                nc.sync.dma_start(xT_s[d][:], xT_d[128 * d:128 * d + 128, :])
            wq_big = wp.tile([128, 8 * F], f32r, tag="wqbig", name="wqbig")
            wk_big = wp.tile([128, 8 * F], f32r, tag="wkbig", name="wkbig")
            wv_big = wp.tile([128, 9 * 260], f32r, tag="wvbig", name="wvbig")
            wq_s = [wq_big[:, F * d:F * d + F] for d in range(8)]
            wk_s = [wk_big[:, F * d:F * d + F] for d in range(8)]
            wv_s = [wv_big[:, 260 * d:260 * d + 260] for d in range(9)]
            for d in range(8):
                nc.sync.dma_start(wq_s[d][:], wqT_d[128 * d:128 * d + 128, :])
                nc.sync.dma_start(wk_s[d][:], wkT_d[128 * d:128 * d + 128, :])
            for d in range(9):
                nc.sync.dma_start(wv_s[d][:], wvTe_d[128 * d:128 * d + 128, :])

            # q^T / k^T : [256, 2048], rope over [128, 1024] half-tiles
            for which, w_s, b_s, dst in (("q", wq_s, qb_s, qhatT),
                                         ("k", wk_s, kb_s, khatT)):
                for fc in range(2):
                    for half in range(2):
                        qs = rp.tile([128, 1024], fp32, tag="qs", name="qs")
                        for jj in range(2):
                            j = 2 * half + jj
                            ps = pp.tile([128, 512], fp32, tag="proj",
                                         name="proj")
                            for d in range(8):
                                MM(
                                    ps[:],
                                    w_s[d][:, 128 * fc:128 * fc + 128],
                                    xT_s[d][:, 512 * j:512 * j + 512],
                                    start=(d == 0), stop=(d == 7),
                                )
                            nc.scalar.activation(
                                qs[:, 512 * jj:512 * jj + 512], ps[:],
                                AF.Identity, bias=b_s[:, fc:fc + 1],
                            )
                        hs = slice(1024 * half, 1024 * half + 1024)
                        sw = rp.tile([128, 1024], fp32, tag="sw", name="sw")
                        for blk, so in enumerate((32, 0, 96, 64)):
                            nc.vector.tensor_copy(sw[32 * blk:32 * blk + 32, :],
                                                  qs[so:so + 32, :])
                        t1 = rp.tile([128, 1024], fp32, tag="t1", name="t1")
                        nc.vector.tensor_mul(t1[:], sw[:], SST_s[:, hs])
                        nc.vector.tensor_mul(qs[:], qs[:], CT_s[:, hs])
                        nc.vector.tensor_add(dst[fc][:, hs], t1[:], qs[:])

            # v token-major [2048, 260] in 16 tiles with bias+ones row fused
            for t in range(16):
                ps = pp.tile([128, 260], fp32, tag="vproj", name="vproj")
                for d in range(9):
                    MM(
                        ps[:],
                        xT_s[d][:, 128 * t:128 * t + 128],
                        wv_s[d][:],
                        start=(d == 0), stop=(d == 8),
                    )
                # evac * padding mask (per-partition = per-token)
                nc.vector.tensor_scalar_mul(v_sb[t][:], ps[:],
                                            mask_s[:, t:t + 1])

        zn_pool = ctx.enter_context(tc.tile_pool(name="zn", bufs=1))
        znT = [zn_pool.tile([128, S], f32r, tag=f"znT{t}", name=f"znT{t}") for t in range(2)]

        # ---------- phase B: attention ----------
        with tc.tile_pool(name="spsum", bufs=3, space="PSUM") as sp, \
             tc.tile_pool(name="zpsum", bufs=2, space="PSUM") as zp, \
             tc.tile_pool(name="pT", bufs=1) as ptp, \
             tc.tile_pool(name="sums", bufs=1) as smp, \
             tc.tile_pool(name="dram", bufs=1, space="DRAM") as dp:
            sums_sb = [smp.tile([1, 8 * 512], fp32, tag=f"sums{p}",
                                name=f"sums{p}") for p in range(2)]
            sums2 = [smp.tile([8, 512], fp32, tag=f"sums2{p}",
                              name=f"sums2{p}") for p in range(2)]
            r_sb = [smp.tile([8, 512], fp32, tag=f"r{p}", name=f"r{p}")
                    for p in range(2)]
            r_dram = [dp.tile([8, 512], fp32, tag=f"rd{p}", name=f"rd{p}")
                      for p in range(2)]
            s_dram = [dp.tile([1, 8 * 512], fp32, tag=f"sd{p}", name=f"sd{p}")
                      for p in range(2)]

            # head-pair software pipeline: scores+exp for both heads of a
            # pair, then each pv accumulation group contiguous on PE (HW
            # has_written constraint: no interleaved psum accum groups).
            for hp in range(2):
                heads = (2 * hp, 2 * hp + 1)
                for j in range(4):
                    pts = {}
                    ni = 4 * j + 4
                    for ip in range((ni + 1) // 2):         # pairs of k-tiles
                      for h in heads:
                        tH, rH = h // 2, 64 * (h % 2)
                        if True:
                            pb = ptp.tile([128, 1024], f32r,
                                          tag=f"p{h % 2}_{ip}", name="p")
                            sps = sp.tile([128, 1024], fp32, tag="s", name="s")
                            cmin = 1024
                            for ii in range(2):
                                i = 2 * ip + ii
                                if i >= ni:
                                    continue
                                d = i - 4 * j
                                c0 = max(0, 128 * d)
                                cmin = min(cmin, 512 * ii + c0)
                                MM(
                                    sps[:, 512 * ii + c0:512 * ii + 512],
                                    khatT[tH][rH:rH + 64,
                                              128 * i:128 * i + 128],
                                    qhatT[tH][rH:rH + 64,
                                              512 * j + c0:512 * j + 512],
                                    start=True, stop=True,
                                )
                                pts[(h, i)] = pb[:, 512 * ii:512 * ii + 512]
                            hi = 1024 if 2 * ip + 1 < ni else 512
                            nc.scalar.activation(pb[:, cmin:hi],
                                                 sps[:, cmin:hi], AF.Exp)
                            for ii in range(2):
                                i = 2 * ip + ii
                                if i >= ni:
                                    continue
                                d = i - 4 * j
                                if d >= 0:
                                    c0 = 512 * ii + max(0, 128 * d)
                                    nc.vector.tensor_mul(pb[:, c0:c0 + 128],
                                                         pb[:, c0:c0 + 128],
                                                         tri_s[:])
                    for h in heads:
                        tH, rH = h // 2, 64 * (h % 2)
                        zps = zp.tile([65, 512], fp32, tag="z", name="z")
                        for i in range(4 * j + 4):
                            d = i - 4 * j
                            c0 = max(0, 128 * d)
                            MM(
                                zps[:, c0:512],
                                v_sb[i][:, 65 * h:65 * h + 65],
                                pts[(h, i)][:, c0:512],
                                start=(i == 0), stop=(i == 4 * j + 3),
                            )
                        row = 4 * (h % 2) + j
                        nc.scalar.copy(
                            sums_sb[hp][0:1, 512 * row:512 * row + 512],
                            zps[64:65, :])
                        nc.vector.tensor_copy(
                            znT[tH][rH:rH + 64, 512 * j:512 * j + 512],
                            zps[0:64, :])

                # normalization for this head pair (overlaps next pair)
                nc.sync.dma_start(s_dram[hp][:], sums_sb[hp][:])
                nc.sync.dma_start(
                    sums2[hp][:],
                    s_dram[hp][0:1, :].rearrange("one (r c) -> (one r) c", r=8))
                nc.vector.reciprocal(r_sb[hp][:], sums2[hp][:])
                nc.sync.dma_start(r_dram[hp][:], r_sb[hp][:])
                for j in range(4):
                    rbc = ptp.tile([128, 512], fp32, tag="rbc", name="rbc")
                    nc.sync.dma_start(
                        rbc[0:64, :],
                        r_dram[hp][j:j + 1, :].to_broadcast((64, 512)))
                    nc.sync.dma_start(
                        rbc[64:128, :],
                        r_dram[hp][4 + j:4 + j + 1, :].to_broadcast((64, 512)))
                    sl = znT[hp][:, 512 * j:512 * j + 512]
                    nc.vector.tensor_mul(sl, sl, rbc[:])
                    if debug and tH == 0 and j == 1:
                        nc.sync.dma_start(dbg_rbc[:], rbc[:])

            if debug:
                for t in range(2):
                    nc.sync.dma_start(dbg_qT[t], qhatT[t][:])
                    nc.sync.dma_start(dbg_kT[t], khatT[t][:])
                    nc.sync.dma_start(dbg_zn[t], znT[t][:])
                nc.sync.dma_start(dbg_v[:], v_big[:])
                nc.sync.dma_start(dbg_sums[:], sums2[:])
                nc.sync.dma_start(dbg_sumsb[:], sums_sb[:])

        # ---------- phase C: output projection ----------
        with tc.tile_pool(name="opsum", bufs=4, space="PSUM") as op, \
             tc.tile_pool(name="osbuf", bufs=4) as ob:
            for m in range(16):
                for n in range(2):
                    ps = op.tile([128, 512], fp32, tag="o", name="o")
                    MM(ps[:], znb_s[:],
                                     wobx_s[:, 512 * n:512 * n + 512],
                                     start=True, stop=False)
                    for t in range(2):
                        MM(
                            ps[:],
                            znT[t][:, 128 * m:128 * m + 128],
                            woT_s[t][:, 512 * n:512 * n + 512],
                            start=False, stop=(t == 1),
                        )
                    ot = ob.tile([128, 512], fp32, tag="osb", name="osb")
                    nc.vector.tensor_copy(ot[:], ps[:])
                    nc.sync.dma_start(
                        out_d[128 * m:128 * m + 128, 512 * n:512 * n + 512], ot[:])

    nc.finalize()
    return nc


_NC_CACHE = {}


_ZNB = np.zeros((128, 128), np.float32)
_ZNB[0, :] = 1.0


def _wobx(wob):
    m = np.zeros((128, DIM), np.float32)
    m[0] = wob
    return m


def kernel(x, attn_mask, wq_w, wq_b, wk_w, wk_b, wv_w, wv_b, wo_w, wo_b):
    from concourse.bass_utils import run_bass_kernel_spmd

    x = np.asarray(x, np.float32)
    attn_mask = np.asarray(attn_mask)
    wq_w = np.asarray(wq_w, np.float32); wq_b = np.asarray(wq_b, np.float32)
    wk_w = np.asarray(wk_w, np.float32); wk_b = np.asarray(wk_b, np.float32)
    wv_w = np.asarray(wv_w, np.float32); wv_b = np.asarray(wv_b, np.float32)
    wo_w = np.asarray(wo_w, np.float32); wo_b = np.asarray(wo_b, np.float32)

    CT, SST = _rope_tables()
    tri01 = np.triu(np.ones((128, 128), np.float32))

    in_maps = []
    for c in range(NC):
        b, g = c // 4, c % 4
        fs = slice(F * g, F * g + F)
        wv = wv_w[fs]
        vb = wv_b[fs]
        wvTe = np.zeros((DIM + 128, 260), np.float32)
        for h in range(HPC):
            wvTe[0:DIM, 65 * h:65 * h + 64] = wv[64 * h:64 * h + 64].T
            wvTe[DIM, 65 * h:65 * h + 64] = vb[64 * h:64 * h + 64]
            wvTe[DIM, 65 * h + 64] = 1.0
        in_maps.append({
            "xT": np.concatenate([x[b].T, np.ones((1, S), np.float32),
                                  np.zeros((127, S), np.float32)], 0).copy(),
            "wqT": np.ascontiguousarray(wq_w[fs].T) / np.float32(8.0),
            "wkT": np.ascontiguousarray(wk_w[fs].T),
            "wvTe": wvTe,
            "woT": np.ascontiguousarray(wo_w[:, fs].T),
            "qb": (wq_b[fs] / np.float32(8.0)).reshape(F, 1).copy(),
            "kb": wk_b[fs].reshape(F, 1).copy(),
            "wob": _wobx(wo_b if g == 0 else np.zeros(DIM, np.float32)),
            "znb": _ZNB,
            "maskv": attn_mask[b].astype(np.float32).reshape(S, 1).copy(),
            "CT": CT, "SST": SST, "tri": tri01,
        })

    if "nc" not in _NC_CACHE:
        _NC_CACHE["nc"] = _build_program()
    res = run_bass_kernel_spmd(_NC_CACHE["nc"], in_maps, core_ids=list(range(NC)))
    globals()["LAST_RESULTS"] = res

    out = np.zeros((B, S, DIM), np.float32)
    for c in range(NC):
        out[c // 4] += res.results[c]["outp"]
    return out


if __name__ == "__main__":
    rng = np.random.default_rng(0)
    ins = {
        "x": rng.standard_normal((B, S, DIM), np.float32),
        "attn_mask": np.ones((B, S), bool),
    }
    for n in ["wq", "wk", "wv", "wo"]:
        ins[n + "_w"] = (rng.standard_normal((DIM, DIM), np.float32) / 32.0)
        ins[n + "_b"] = rng.standard_normal(DIM, np.float32) * 0.01
    o = kernel(**ins)
    print("ran", o.shape, o.dtype)



# revision 13
# speedup vs baseline: 1.1149x; 1.1149x over previous
"""AttentionWithRope Trainium2 Bass kernel (v2, fp16 matmul datapath).

Sharding: 8 cores = 2 batches x 4 head-groups (4 heads / 256 features).
Per core: project q/k/v (fp16 weights+activations, fp32 psum), RoPE via
stream_shuffle (head features host-permuted so the rotate-half pair sits
+-16 partitions apart inside a 32-partition shuffle window), causal
attention for 4 heads with additive -1e30 pre-exp masking, transposed
output projection (dims on partitions). Host sums the 4 partial outT per
batch and adds wo_b.

Engine split: PE matmuls fp16; ACT does exp only (1024-wide tiles);
DVE does shuffle + (q+b)*CT + reciprocal + z-normalize; Pool does
(sw+bsw)*SST + rope-add + v evac + diag mask-add + half of outT evac.

Program is chunk-pipelined over 512-token q-chunks: x DMA, projections,
attention, and output projection for chunk j overlap chunk j+1.
"""

import numpy as np
from contextlib import ExitStack

DIM, HEADS, HD = 1024, 16, 64
B, S = 2, 2048
NC = 8
HPC = 4          # heads per core
F = HPC * HD     # 256 features per core
ROPE_BASE = 10000.0
NEG = np.float32(-1e30)


def _head_perm():
    # within-head feature order: [x1[0:16], x2[0:16], x1[16:32], x2[16:32]]
    return np.concatenate([np.arange(0, 16), np.arange(32, 48),
                           np.arange(16, 32), np.arange(48, 64)])


def _rope_tables():
    """CT/SST [128, S] fp32 in the permuted layout (2 heads per tile)."""
    theta = ROPE_BASE ** (-np.arange(0, HD, 2, dtype=np.float32) / HD)  # [32]
    pos = np.arange(S, dtype=np.float32)
    ang = pos[:, None] * theta[None, :]            # [S, 32]
    cos, sin = np.cos(ang).T, np.sin(ang).T        # [32, S]
    cA, cB = cos[0:16], cos[16:32]
    sA, sB = sin[0:16], sin[16:32]
    ct_head = np.concatenate([cA, cA, cB, cB], 0)              # [64, S]
    sst_head = np.concatenate([-sA, sA, -sB, sB], 0)           # [64, S]
    CT = np.concatenate([ct_head, ct_head], 0).astype(np.float32)    # [128, S]
    SST = np.concatenate([sst_head, sst_head], 0).astype(np.float32)
    return CT, SST


_SHUF_MASK = list(range(16, 32)) + list(range(0, 16))


def _build_program(debug=False):
    import concourse.bass as bass
    import concourse.mybir as mybir
    import concourse.tile as tile
    from concourse import bacc

    fp32 = mybir.dt.float32
    f16 = mybir.dt.float16
    AF = mybir.ActivationFunctionType
    ALU = mybir.AluOpType

    nc = bacc.Bacc("TRN2", target_bir_lowering=False, num_devices=NC)

    from bass_rust import add_dep_helper as _adh
    _prev_mm = [None]

    def MM(*args, **kw):
        bi = nc.tensor.matmul(*args, **kw)
        if _prev_mm[0] is not None:
            _adh(bi.ins, _prev_mm[0].ins, sync=False, reason="pe-order")
        _prev_mm[0] = bi
        return bi

    # ---- DRAM I/O ----
    xT_d = nc.dram_tensor("xT", [DIM + 128, S], f16, kind="ExternalInput").ap()
    wqT_d = nc.dram_tensor("wqT", [DIM + 128, F], f16, kind="ExternalInput").ap()
    wkT_d = nc.dram_tensor("wkT", [DIM + 128, F], f16, kind="ExternalInput").ap()
    wvTe_d = nc.dram_tensor("wvTe", [DIM + 128, 260], f16, kind="ExternalInput").ap()
    woT_d = nc.dram_tensor("woT", [F, DIM], f16, kind="ExternalInput").ap()
    mask_d = nc.dram_tensor("maskv", [S, 1], fp32, kind="ExternalInput").ap()
    CT_d = nc.dram_tensor("CT", [128, S], fp32, kind="ExternalInput").ap()
    SST_d = nc.dram_tensor("SST", [128, S], fp32, kind="ExternalInput").ap()
    tri_d = nc.dram_tensor("tri", [128, 128], f16, kind="ExternalInput").ap()
    negI_d = nc.dram_tensor("negI", [128, 128], f16, kind="ExternalInput").ap()
    out_d = nc.dram_tensor("outp", [DIM, S], fp32, kind="ExternalOutput").ap()
    if debug:
        dbg_qT = nc.dram_tensor("dbg_qT", [2, 128, S], f16, kind="ExternalOutput").ap()
        dbg_kT = nc.dram_tensor("dbg_kT", [2, 128, S], f16, kind="ExternalOutput").ap()
        dbg_v = nc.dram_tensor("dbg_v", [128, 16 * 260], f16, kind="ExternalOutput").ap()
        dbg_zn = nc.dram_tensor("dbg_zn", [2, 128, S], f16, kind="ExternalOutput").ap()

    with tile.TileContext(nc) as tc, ExitStack() as ctx:
        # ---------- persistent SBUF ----------
        const = ctx.enter_context(tc.tile_pool(name="const", bufs=1))
        qk_pool = ctx.enter_context(tc.tile_pool(name="qk", bufs=1))
        v_pool = ctx.enter_context(tc.tile_pool(name="v", bufs=1))
        zn_pool = ctx.enter_context(tc.tile_pool(name="zn", bufs=1))

        tri_s = const.tile([128, 128], f16, tag="tri", name="tri")
        negI_s = const.tile([128, 128], f16, tag="negI", name="negI")
        nc.sync.dma_start(negI_s[:], negI_d[:])
        mask_s = const.tile([128, 16], fp32, tag="maskv", name="maskv")
        CT_s = const.tile([128, S], fp32, tag="CT", name="CT")
        SST_s = const.tile([128, S], fp32, tag="SST", name="SST")
        woT_s = [const.tile([128, DIM], f16, tag=f"woT{t}", name=f"woT{t}")
                 for t in range(2)]
        nc.sync.dma_start(tri_s[:], tri_d[:])
        nc.sync.dma_start(mask_s[:], mask_d.rearrange("(t p) one -> p (t one)", p=128))
        nc.sync.dma_start(CT_s[:], CT_d[:])
        nc.sync.dma_start(SST_s[:], SST_d[:])
        for t in range(2):
            nc.sync.dma_start(woT_s[t][:], woT_d[128 * t:128 * t + 128, :])

        # weights
        wq_big = const.tile([128, 9 * F], f16, tag="wqbig", name="wqbig")
        wk_big = const.tile([128, 9 * F], f16, tag="wkbig", name="wkbig")
        wv_big = const.tile([128, 9 * 260], f16, tag="wvbig", name="wvbig")
        wq_s = [wq_big[:, F * d:F * d + F] for d in range(9)]
        wk_s = [wk_big[:, F * d:F * d + F] for d in range(9)]
        wv_s = [wv_big[:, 260 * d:260 * d + 260] for d in range(9)]
        for d in range(9):
            nc.sync.dma_start(wq_s[d][:], wqT_d[128 * d:128 * d + 128, :])
            nc.sync.dma_start(wk_s[d][:], wkT_d[128 * d:128 * d + 128, :])
            nc.sync.dma_start(wv_s[d][:], wvTe_d[128 * d:128 * d + 128, :])

        # x, chunked by 512-token column groups so chunk 0 lands first
        xT_big = const.tile([128, 9 * S], f16, tag="xTbig", name="xTbig")
        xT_s = [xT_big[:, S * d:S * d + S] for d in range(9)]
        for j in range(4):
            cs = slice(512 * j, 512 * j + 512)
            for d in range(9):
                nc.sync.dma_start(xT_s[d][:, cs], xT_d[128 * d:128 * d + 128, cs])

        qhatT = [qk_pool.tile([128, S], f16, tag=f"qhatT{t}", name=f"qhatT{t}")
                 for t in range(2)]
        khatT = [qk_pool.tile([128, S], f16, tag=f"khatT{t}", name=f"khatT{t}")
                 for t in range(2)]
        v_big = v_pool.tile([128, 16 * 260], f16, tag="vbig", name="vbig")
        v_sb = [v_big[:, 260 * t:260 * t + 260] for t in range(16)]
        znT = [zn_pool.tile([128, S], f16, tag=f"znT{t}", name=f"znT{t}")
               for t in range(2)]

        pp = ctx.enter_context(tc.tile_pool(name="pp", bufs=2, space="PSUM"))
        sp = ctx.enter_context(tc.tile_pool(name="sp", bufs=2, space="PSUM"))
        zp = ctx.enter_context(tc.tile_pool(name="zp", bufs=2, space="PSUM"))
        rope_p = ctx.enter_context(tc.tile_pool(name="ropet", bufs=3))
        pb_p = ctx.enter_context(tc.tile_pool(name="pb", bufs=1))
        rr_p = ctx.enter_context(tc.tile_pool(name="rr", bufs=4))
        ob_p = ctx.enter_context(tc.tile_pool(name="ob", bufs=4))
        dp = ctx.enter_context(tc.tile_pool(name="dram", bufs=4, space="DRAM"))

        pb_big = pb_p.tile([128, 16 * 1024], f16, tag="pbbig", name="pbbig")

        def out_proj(j):
            cs = slice(512 * j, 512 * j + 512)
            for n in range(8):
                ps = pp.tile([128, 512], fp32, tag="proj", name="proj")
                for t in range(2):
                    MM(ps[:], woT_s[t][:, 128 * n:128 * n + 128],
                       znT[t][:, cs], start=(t == 0), stop=(t == 1))
                ot = ob_p.tile([128, 512], fp32, tag="osb", name="osb")
                if n % 2 == 0:
                    nc.vector.tensor_copy(ot[:], ps[:])
                else:
                    nc.scalar.copy(ot[:], ps[:])
                nc.sync.dma_start(out_d[128 * n:128 * n + 128, cs], ot[:])

        for j in range(4):
            cs = slice(512 * j, 512 * j + 512)
            # ---- q/k projections + rope for chunk j ----
            for which, w_s, dst in (("q", wq_s, qhatT), ("k", wk_s, khatT)):
                for fc in range(2):
                    ps = pp.tile([128, 512], fp32, tag="proj", name="proj")
                    for d in range(9):
                        MM(ps[:], w_s[d][:, 128 * fc:128 * fc + 128],
                           xT_s[d][:, cs], start=(d == 0), stop=(d == 8))
                    # rope: dst = ps*CT + shuffle(ps)*SST  (bias already in ps)
                    sw = rope_p.tile([128, 512], fp32, tag="sw", name="sw")
                    nc.vector.stream_shuffle(sw[:], ps[:], _SHUF_MASK)
                    t1 = rope_p.tile([128, 512], fp32, tag="t1", name="t1")
                    nc.gpsimd.tensor_mul(t1[:], sw[:], SST_s[:, cs])
                    t2 = rope_p.tile([128, 512], fp32, tag="t2", name="t2")
                    nc.vector.tensor_mul(t2[:], ps[:], CT_s[:, cs])
                    nc.gpsimd.tensor_add(dst[fc][:, cs], t1[:], t2[:])

            # ---- v projection for token tiles of chunk j ----
            for t in range(4 * j, 4 * j + 4):
                ps = pp.tile([128, 512], fp32, tag="proj", name="proj")
                for d in range(9):
                    MM(ps[:, 0:260], xT_s[d][:, 128 * t:128 * t + 128],
                       wv_s[d][:], start=(d == 0), stop=(d == 8))
                nc.scalar.activation(v_sb[t][:], ps[:, 0:260], AF.Identity,
                                     scale=mask_s[:, t:t + 1])

            # ---- output projection for the previous chunk ----
            if j > 0:
                out_proj(j - 1)

            # ---- attention for chunk j, 4 heads ----
            ni = 4 * j + 4
            for h in range(HPC):
                tH, rH = h // 2, 64 * (h % 2)
                pts = {}
                for ip in range((ni + 1) // 2):
                    sps = sp.tile([128, 1024], fp32, tag="s", name="s")
                    slot = 8 * (h % 2) + ip
                    pb = pb_big[:, 1024 * slot:1024 * slot + 1024]
                    cmin = 1024
                    for ii in range(2):
                        i = 2 * ip + ii
                        if i >= ni:
                            continue
                        d = i - 4 * j
                        c0 = max(0, 128 * d)
                        cmin = min(cmin, 512 * ii + c0)
                        MM(sps[:, 512 * ii + c0:512 * ii + 512],
                           khatT[tH][rH:rH + 64, 128 * i:128 * i + 128],
                           qhatT[tH][rH:rH + 64, 512 * j + c0:512 * j + 512],
                           start=True, stop=(d < 0))
                        pts[i] = pb[:, 512 * ii:512 * ii + 512]
                        if d >= 0:
                            # additive causal mask on the diagonal block via
                            # PE: accumulate (-60000*I)^T @ tril01
                            cc = 512 * ii + 128 * d
                            MM(sps[:, cc:cc + 128], negI_s[:], tri_s[:],
                               start=False, stop=True)
                    hi = 1024 if 2 * ip + 1 < ni else 512
                    nc.scalar.activation(pb[:, cmin:hi], sps[:, cmin:hi], AF.Exp)

                zps = zp.tile([128, 512], fp32, tag="z", name="z")
                for i in range(ni):
                    d = i - 4 * j
                    c0 = max(0, 128 * d)
                    MM(zps[0:65, c0:512], v_sb[i][:, 65 * h:65 * h + 65],
                       pts[i][:, c0:512], start=(i == 0), stop=(i == ni - 1))
                # normalize: r = 1/sums, broadcast via DRAM, fused evac
                r_sb = rr_p.tile([1, 512], fp32, tag="r", name="r")
                nc.vector.reciprocal(r_sb[:], zps[64:65, 0:512])
                r_dram = dp.tile([1, 512], fp32, tag="rd", name="rd")
                nc.sync.dma_start(r_dram[:], r_sb[:])
                rbc = rr_p.tile([64, 512], fp32, tag="rbc", name="rbc")
                nc.sync.dma_start(rbc[:], r_dram[0:1, :].to_broadcast((64, 512)))
                if rH == 0:
                    nc.vector.tensor_mul(znT[tH][0:64, cs], zps[0:64, 0:512],
                                         rbc[:])
                else:
                    tmp = rope_p.tile([64, 512], fp32, tag="ztmp", name="ztmp")
                    nc.vector.tensor_mul(tmp[:], zps[0:64, 0:512], rbc[:])
                    nc.gpsimd.tensor_copy(znT[tH][64:128, cs], tmp[:])

        # ---- output projection for the final chunk ----
        out_proj(3)

        if debug:
            for t in range(2):
                nc.sync.dma_start(dbg_qT[t], qhatT[t][:])
                nc.sync.dma_start(dbg_kT[t], khatT[t][:])
                nc.sync.dma_start(dbg_zn[t], znT[t][:])
            nc.sync.dma_start(dbg_v[:], v_big[:])

    nc.finalize()
    return nc


_NC_CACHE = {}


def kernel(x, attn_mask, wq_w, wq_b, wk_w, wk_b, wv_w, wv_b, wo_w, wo_b):
    from concourse.bass_utils import run_bass_kernel_spmd

    x = np.asarray(x, np.float32)
    attn_mask = np.asarray(attn_mask)
    wq_w = np.asarray(wq_w, np.float32); wq_b = np.asarray(wq_b, np.float32)
    wk_w = np.asarray(wk_w, np.float32); wk_b = np.asarray(wk_b, np.float32)
    wv_w = np.asarray(wv_w, np.float32); wv_b = np.asarray(wv_b, np.float32)
    wo_w = np.asarray(wo_w, np.float32); wo_b = np.asarray(wo_b, np.float32)

    CT, SST = _rope_tables()
    # causal mask helpers: diag-block mask via PE accumulation of
    # (-60000*I)^T @ tril01 -> exp underflows to exactly 0 where k > q
    tri01 = np.tril(np.ones((128, 128), np.float16), -1)
    negI = (np.eye(128) * -60000.0).astype(np.float16)

    hp = _head_perm()
    perm = np.concatenate([64 * h + hp for h in range(HPC)])  # [256]
    shuf = np.concatenate([32 * w + np.array(_SHUF_MASK) for w in range(8)])

    in_maps = []
    for c in range(NC):
        b, g = c // 4, c % 4
        fs = slice(F * g, F * g + F)
        wq = wq_w[fs][perm] / np.float32(8.0)
        wk = wk_w[fs][perm]
        qb = wq_b[fs][perm] / np.float32(8.0)
        kb = wk_b[fs][perm]
        wqTe = np.zeros((DIM + 128, F), np.float16)
        wqTe[0:DIM] = wq.T.astype(np.float16)
        wqTe[DIM] = qb.astype(np.float16)
        wkTe = np.zeros((DIM + 128, F), np.float16)
        wkTe[0:DIM] = wk.T.astype(np.float16)
        wkTe[DIM] = kb.astype(np.float16)
        wv = wv_w[fs]
        vb = wv_b[fs]
        wvTe = np.zeros((DIM + 128, 260), np.float16)
        for h in range(HPC):
            wvTe[0:DIM, 65 * h:65 * h + 64] = wv[64 * h:64 * h + 64].T.astype(np.float16)
            wvTe[DIM, 65 * h:65 * h + 64] = vb[64 * h:64 * h + 64].astype(np.float16)
            wvTe[DIM, 65 * h + 64] = 1.0
        xTe = np.zeros((DIM + 128, S), np.float16)
        xTe[0:DIM] = x[b].T.astype(np.float16)
        xTe[DIM] = 1.0
        in_maps.append({
            "xT": xTe,
            "wqT": wqTe,
            "wkT": wkTe,
            "wvTe": wvTe,
            "woT": np.ascontiguousarray(wo_w[:, fs].T).astype(np.float16),
            "maskv": attn_mask[b].astype(np.float32).reshape(S, 1).copy(),
            "CT": CT, "SST": SST, "tri": tri01, "negI": negI,
        })

    if "nc" not in _NC_CACHE:
        _NC_CACHE["nc"] = _build_program()
    res = run_bass_kernel_spmd(_NC_CACHE["nc"], in_maps, core_ids=list(range(NC)))
    globals()["LAST_RESULTS"] = res

    out = np.zeros((B, DIM, S), np.float32)
    for c in range(NC):
        out[c // 4] += res.results[c]["outp"]
    out = out.transpose(0, 2, 1) + wo_b[None, None, :]
    return np.ascontiguousarray(out)


if __name__ == "__main__":
    rng = np.random.default_rng(0)
    ins = {
        "x": rng.standard_normal((B, S, DIM)).astype(np.float32),
        "attn_mask": np.ones((B, S), bool),
    }
    for n in ["wq", "wk", "wv", "wo"]:
        ins[n + "_w"] = (rng.standard_normal((DIM, DIM)).astype(np.float32) / 32.0)
        ins[n + "_b"] = rng.standard_normal(DIM).astype(np.float32) * 0.01
    o = kernel(**ins)
    print("ran", o.shape, o.dtype)
